# revision 8
# baseline (speedup 1.0000x reference)
"""Multi-head attention (B=4, S=2048, D=1024, H=16) on 8 Trainium2 cores.

Sharding (v6): core c -> head-pair p = c (2 heads, 128 output dims), all 4
batches.  This makes the valid_len truncation SPMD-uniform: every core runs
the same per-batch k-loop trip counts kc_b = ceil(valid_len[b]/128), so the
attention work beyond the key-padding boundary is simply never emitted.
W_o is row-split by head-pair; each core emits a full-shape [B, S, D] fp16
partial and the host sums the 8 partials.

Masking: the host zeroes xv columns at k >= valid_len[b] and supplies a
masked ones-column (stored last in each V tile), so masked keys contribute
exactly 0 to both the attention*V accumulation and the softmax denominator.

Device layout notes:
  - matmul computes lhsT.T @ rhs with contraction on the partition dim.
  - Q/K are produced transposed ([dout, s]) so scores come out as
    scores^T [k, q]; 1/sqrt(hd) folds into the ScalarE exp op's scale.
    The two heads' scores matmuls use PE row-groups 0-63 / 64-127 and run
    concurrently.
  - All matmul inputs fp16 (4x faster PE than fp32), fp32 PSUM accumulation.
  - ScalarE runs exp ONLY; every PSUM drain is on VectorE so the ACT-bound
    attention pipeline is as short as possible.
  - PSUM: scp 2x2 banks + av 2 + one uniform fill tag (2x1 banks) = 8.
    Q/K/V projection of batch b+1 and O-projection of batch b fill the PE
    gaps of the ACT-bound attention pipeline of batch b.
  - DMA: inputs stream as full-row tiles (>=2KB packets); output written as
    [128, 1024] f16 tiles (2KB rows) to keep DMA dispatch count low.
"""

import contextlib

import numpy as np

import concourse.bacc as bacc
import concourse.mybir as mybir
import concourse.tile as tile
from concourse.bass_utils import run_bass_kernel_spmd

F16 = mybir.dt.float16
F32 = mybir.dt.float32
AF = mybir.ActivationFunctionType

B, S, D, H, HD = 4, 2048, 1024, 16, 64
NQB = S // 512        # query blocks of 512

_cache = {}


def _qk_proj_b(nc, b, kcb, ap, stream, fill, wq_sb, wk_sb, qT_sb, kT_sb):
    """Q/K projection for one batch (both heads of the pair = 128 dims)."""
    xq_t = []
    for dj in range(8):
        xqt = stream.tile([128, S], F16, tag="xq", name="xqt", bufs=9)
        nc.sync.dma_start(xqt[:], ap[f"xq{b}"][dj * 128:(dj + 1) * 128, :])
        xq_t.append(xqt)
    for qb in range(NQB):
        psq = fill.tile([128, 512], F32, tag="fill", name="psq", bufs=2)
        for dj in range(8):
            nc.tensor.matmul(psq[:], wq_sb[dj][:],
                             xq_t[dj][:, qb * 512:(qb + 1) * 512],
                             start=(dj == 0), stop=(dj == 7))
        nc.vector.tensor_copy(qT_sb[b][:, qb * 512:(qb + 1) * 512], psq[:])
    nk = kcb * 128
    for r0 in range(0, nk, 1024):
        n2 = min(1024, nk - r0)
        xk_t = []
        for dj in range(8):
            xkt = stream.tile([128, 1024], F16, tag="xk", name="xkt", bufs=9)
            nc.sync.dma_start(xkt[:, 0:n2],
                              ap[f"xk{b}"][dj * 128:(dj + 1) * 128,
                                           r0:r0 + n2])
            xk_t.append(xkt)
        for c0 in range(0, n2, 512):
            n = min(512, n2 - c0)
            psk = fill.tile([128, 512], F32, tag="fill", name="psk", bufs=2)
            for dj in range(8):
                nc.tensor.matmul(psk[:, 0:n], wk_sb[dj][:],
                                 xk_t[dj][:, c0:c0 + n],
                                 start=(dj == 0), stop=(dj == 7))
            nc.vector.tensor_copy(
                kT_sb[b][:, r0 + c0:r0 + c0 + n], psk[:, 0:n])


def _v_proj_b(nc, b, kcb, ap, stream, fill, wv_sb, v_sb, vm_sb):
    """V projection for one batch, natural [s, dout].  v_sb[b][sc] is
    [128, 2, HD+1] fp16 with the host-masked ones column last."""
    nk = kcb * 128
    for r0 in range(0, nk, 1024):
        n2 = min(1024, nk - r0)
        xv_t = []
        for dj in range(8):
            xvt = stream.tile([128, 1024], F16, tag="xv", name="xvt", bufs=9)
            nc.sync.dma_start(xvt[:, 0:n2],
                              ap[f"xv{b}"][dj * 128:(dj + 1) * 128,
                                           r0:r0 + n2])
            xv_t.append(xvt)
        for sc in range(r0 // 128, (r0 + n2) // 128):
            off = sc * 128 - r0
            psv = fill.tile([128, 512], F32, tag="fill", name="psv", bufs=2)
            for dj in range(8):
                nc.tensor.matmul(psv[:, 0:128],
                                 xv_t[dj][:, off:off + 128],
                                 wv_sb[dj][:], start=(dj == 0),
                                 stop=(dj == 7))
            nc.vector.tensor_copy(v_sb[b][sc][:, :, 0:HD], psv[:, 0:128])
            nc.vector.tensor_copy(
                v_sb[b][sc][:, :, HD], vm_sb[:, b * 16 + sc, :])


def _attention_b(nc, b, kcb, psum, expool, wrk, qT_sb, kT_sb, v_sb, ctx_sb):
    """Attention for one batch (2 heads): ACT-bound pipeline over (qb, kc).
    ScalarE does exp only; all other work is on VectorE/GpSimd/PE."""
    for qb in range(NQB):
        av = psum.tile([HD + 1, 2, 512], F32, tag="av", name="av")
        for kc in range(kcb):
            scp = psum.tile([128, 2, 512], F32, tag="sc", name="scp", bufs=2)
            for h2 in range(2):
                nc.tensor.matmul(
                    scp[:, h2, :],
                    kT_sb[b][64 * h2:64 * h2 + 64, kc * 128:(kc + 1) * 128],
                    qT_sb[b][64 * h2:64 * h2 + 64, qb * 512:(qb + 1) * 512],
                    start=True, stop=True)
            ex = expool.tile([128, 2, 512], F16, tag="ex", name="ex")
            nc.scalar.activation(ex[:], scp[:], AF.Exp, scale=0.125)
            for h2 in range(2):
                nc.tensor.matmul(
                    av[:, h2, :], v_sb[b][kc][:, h2, :], ex[:, h2, :],
                    start=(kc == 0), stop=(kc == kcb - 1))
        # Drain av to SBUF (frees the PSUM slot), then normalize off the
        # critical path: ctx[m, q] = av[m, q] / av[64, q].
        avc = wrk.tile([HD + 1, 2, 512], F32, tag="avc", name="avc", bufs=2)
        nc.vector.tensor_copy(avc[:], av[:])
        # Move the sums row to partition 0 (engines can't read from a
        # non-32-aligned base partition; partition_broadcast reads p0 only).
        r0 = wrk.tile([1, 1024], F32, tag="r0", name="r0", bufs=2)
        nc.sync.dma_start(r0[:], avc[HD:HD + 1, :, :])
        bc = wrk.tile([HD, 1024], F32, tag="bc", name="bc", bufs=2)
        nc.gpsimd.partition_broadcast(bc[:], r0[0:1, :])
        recb = wrk.tile([HD, 1024], F32, tag="recb", name="recb", bufs=2)
        nc.vector.reciprocal_approx_fast(recb[:], bc[:])
        nc.vector.tensor_mul(
            ctx_sb[b][0:HD, qb * 512:(qb + 1) * 512],
            avc[0:HD, 0, :], recb[:, 0:512])
        tmp = wrk.tile([HD, 512], F16, tag="tmpb", name="tmp", bufs=2)
        nc.vector.tensor_mul(tmp[:], avc[0:HD, 1, :], recb[:, 512:1024])
        nc.sync.dma_start(
            ctx_sb[b][HD:128, qb * 512:(qb + 1) * 512], tmp[:])


def _o_proj_b(nc, b, ap, fill, wrk, ctx_sb, wo_sb):
    """Output projection partial for one batch; [128, 1024] f16 staging
    tiles keep the output DMA rows at 2KB."""
    for sc in range(16):
        ot = wrk.tile([128, 1024], F16, tag="ot", name="ot", bufs=4)
        for ih in range(2):
            po = fill.tile([128, 512], F32, tag="fill", name="po", bufs=2)
            nc.tensor.matmul(
                po[:], ctx_sb[b][:, sc * 128:(sc + 1) * 128],
                wo_sb[0][:, ih * 512:(ih + 1) * 512],
                start=True, stop=True)
            nc.vector.tensor_copy(ot[:, ih * 512:(ih + 1) * 512], po[:])
        nc.sync.dma_start(
            ap["out"][b, sc * 128:(sc + 1) * 128, :], ot[:])


def _emit(nc, tc, ap, kcs):
    es = contextlib.ExitStack()
    with es:
        const = es.enter_context(tc.tile_pool(name="const", bufs=1))
        resid = es.enter_context(tc.tile_pool(name="resid", bufs=1))
        stream = es.enter_context(tc.tile_pool(name="stream", bufs=3))
        expool = es.enter_context(tc.tile_pool(name="expool", bufs=3))
        wrk = es.enter_context(tc.tile_pool(name="wrk", bufs=2))

        # constants: per-dj [din-chunk, dout=128] weight tiles for the pair
        wq_sb = [const.tile([128, 128], F16, tag=f"wq{i}", name=f"wq{i}")
                 for i in range(8)]
        wk_sb = [const.tile([128, 128], F16, tag=f"wk{i}", name=f"wk{i}")
                 for i in range(8)]
        wv_sb = [const.tile([128, 128], F16, tag=f"wv{i}", name=f"wv{i}")
                 for i in range(8)]
        wo_sb = [const.tile([128, D], F16, tag="wo", name="wo")]
        vm_sb = const.tile([128, 64, 2], F16, tag="vmask", name="vmask")
        nc.sync.dma_start(vm_sb[:], ap["vones"])
        for i in range(8):
            nc.sync.dma_start(wq_sb[i][:], ap["wq"][i * 128:(i + 1) * 128, :])
            nc.sync.dma_start(wk_sb[i][:], ap["wk"][i * 128:(i + 1) * 128, :])
            nc.sync.dma_start(wv_sb[i][:], ap["wv"][i * 128:(i + 1) * 128, :])
        nc.sync.dma_start(wo_sb[0][:], ap["wo"])

        # residents (per batch)
        qT_sb = [resid.tile([128, S], F16, tag=f"qT{b}", name=f"qT{b}")
                 for b in range(B)]
        kT_sb = [resid.tile([128, kcs[b] * 128], F16, tag=f"kT{b}",
                            name=f"kT{b}") for b in range(B)]
        ctx_sb = [resid.tile([128, S], F16, tag=f"ctx{b}", name=f"ctx{b}")
                  for b in range(B)]
        v_sb = [[resid.tile([128, 2, HD + 1], F16, tag=f"v{b}_{i}",
                            name=f"v{b}_{i}") for i in range(kcs[b])]
                for b in range(B)]

        # Fill pool (2x1 bank, uniform tag) carries Q/K/V projections and
        # the O-projection; they run in the PE gaps of the ACT-bound
        # attention.
        order = sorted(range(B), key=lambda b: -kcs[b])
        with tc.tile_pool(name="fill_psum", bufs=1, space="PSUM") as fill:
            b0 = order[0]
            _qk_proj_b(nc, b0, kcs[b0], ap, stream, fill,
                       wq_sb, wk_sb, qT_sb, kT_sb)
            _v_proj_b(nc, b0, kcs[b0], ap, stream, fill, wv_sb, v_sb, vm_sb)
            with tc.tile_pool(name="at_psum", bufs=1, space="PSUM") as at_psum:
                # attention first = higher scheduler priority; projections and
                # O fill the PE gaps of the ACT-bound pipeline.
                for i, b in enumerate(order):
                    _attention_b(nc, b, kcs[b], at_psum, expool, wrk,
                                 qT_sb, kT_sb, v_sb, ctx_sb)
                    if i + 1 < B:
                        nb = order[i + 1]
                        _qk_proj_b(nc, nb, kcs[nb], ap, stream, fill,
                                   wq_sb, wk_sb, qT_sb, kT_sb)
                        _v_proj_b(nc, nb, kcs[nb], ap, stream, fill,
                                  wv_sb, v_sb, vm_sb)
                    _o_proj_b(nc, b, ap, fill, wrk, ctx_sb, wo_sb)


def _build(kcs):
    key = ("nc", tuple(kcs))
    if key in _cache:
        return _cache[key]
    nc = bacc.Bacc("TRN2", target_bir_lowering=False, debug=False, num_devices=8)
    ap = {"wq": nc.dram_tensor("wq", [D, 128], F16, kind="ExternalInput").ap(),
          "wk": nc.dram_tensor("wk", [D, 128], F16, kind="ExternalInput").ap(),
          "wv": nc.dram_tensor("wv", [D, 128], F16, kind="ExternalInput").ap(),
          "wo": nc.dram_tensor("wo", [128, D], F16, kind="ExternalInput").ap(),
          "vones": nc.dram_tensor("vones", [128, 64, 2], F16,
                                  kind="ExternalInput").ap(),
          "out": nc.dram_tensor("out", [B, S, D], F16,
                                kind="ExternalOutput").ap()}
    for b in range(B):
        ap[f"xq{b}"] = nc.dram_tensor(f"xq{b}", [D, S], F16,
                                      kind="ExternalInput").ap()
        ap[f"xk{b}"] = nc.dram_tensor(f"xk{b}", [D, kcs[b] * 128], F16,
                                      kind="ExternalInput").ap()
        ap[f"xv{b}"] = nc.dram_tensor(f"xv{b}", [D, kcs[b] * 128], F16,
                                      kind="ExternalInput").ap()
    with tile.TileContext(nc) as tc:
        _emit(nc, tc, ap, kcs)
    nc.compile()
    _cache[key] = nc
    return nc


def _in_maps(kcs, queries, keys, values, valid_len, W_q, W_k, W_v, W_o):
    f16 = np.float16
    # host-masked ones column: 1 where k < valid_len[b], else 0
    # vones[p, b*16+sc, h] = 1 if sc*128+p < valid_len[b] else 0
    kpos = np.arange(16 * 128).reshape(16, 128)
    vones = np.zeros((128, 64, 2), f16)
    for b in range(B):
        v1 = (kpos < int(valid_len[b])).astype(f16)  # [16, 128]
        vones[:, b * 16:(b + 1) * 16, :] = v1.T[:, :, None]
    maps = []
    for c in range(8):
        j0 = 128 * c
        m = {
            "wq": np.ascontiguousarray(W_q[j0:j0 + 128, :].T).astype(f16),
            "wk": np.ascontiguousarray(W_k[j0:j0 + 128, :].T).astype(f16),
            "wv": np.ascontiguousarray(W_v[j0:j0 + 128, :].T).astype(f16),
            "wo": np.ascontiguousarray(W_o[:, j0:j0 + 128].T).astype(f16),
            "vones": vones,
        }
        for b in range(B):
            nk = kcs[b] * 128
            xv = values[b][:nk].T.copy()      # [D, nk]
            xv[:, int(valid_len[b]):] = 0.0   # mask padding rows of v
            m[f"xq{b}"] = np.ascontiguousarray(queries[b].T).astype(f16)
            m[f"xk{b}"] = np.ascontiguousarray(keys[b][:nk].T).astype(f16)
            m[f"xv{b}"] = xv.astype(f16)
        maps.append(m)
    return maps


def kernel(queries, keys, values, valid_len, W_q, W_k, W_v, W_o, _run_kwargs=None):
    queries = np.asarray(queries, np.float32)
    keys = np.asarray(keys, np.float32)
    values = np.asarray(values, np.float32)
    valid_len = np.asarray(valid_len)
    W_q = np.asarray(W_q, np.float32)
    W_k = np.asarray(W_k, np.float32)
    W_v = np.asarray(W_v, np.float32)
    W_o = np.asarray(W_o, np.float32)

    kcs = [max(1, min(16, -(-int(valid_len[b]) // 128))) for b in range(B)]
    nc = _build(kcs)
    maps = _in_maps(kcs, queries, keys, values, valid_len, W_q, W_k, W_v, W_o)
    res = run_bass_kernel_spmd(nc, maps, list(range(8)), **(_run_kwargs or {}))
    out = np.zeros((B, S, D), np.float32)
    for c in range(8):
        out += res.results[c]["out"].astype(np.float32)
    if _run_kwargs:
        _cache["last_results"] = res
    return out


# revision 19
# speedup vs baseline: 1.0505x; 1.0505x over previous
"""Multi-head attention (B=4, S=2048, D=1024, H=16) on 8 Trainium2 cores.

Sharding (v7): core c -> head-pair p = c (2 heads, 128 output dims), all 4
batches.  This makes the valid_len truncation SPMD-uniform: every core runs
the same per-batch k-loop trip counts kc_b = ceil(valid_len[b]/128), so the
attention work beyond the key-padding boundary is simply never emitted.
W_o is row-split by head-pair; each core emits a full-shape [B, S, D] fp16
partial and the host sums the 8 partials.

Masking: the host zeroes xv columns at k >= valid_len[b] and supplies a
masked ones-column (stored last in each V tile), so masked keys contribute
exactly 0 to both the attention*V accumulation and the softmax denominator.

Scheduling (the v7 point): the attention pipeline is ACT(exp)-bound at
~1.15us per kc iteration while its PE demand is only ~0.65us, and the HAM
power manager throttles the PE to ~50% rate whenever the stream has gaps.
So ALL non-attention PE work (Q/K/V projection chains, O-projection) is
emitted through a single global fill GENERATOR that the attention loop
pulls ~500ns-sized quanta from after every kc iteration.  Each batch's own
K/V/Q tails are pipelined into its own attention window (only qb0/kc-round0
is emitted eagerly as prologue), and O(b) fills the next batch's window.

Device layout notes:
  - matmul computes lhsT.T @ rhs with contraction on the partition dim.
  - Q/K are produced transposed ([dout, s]) so scores come out as
    scores^T [k, q]; 1/sqrt(hd) folds into the ScalarE exp op's scale.
    The two heads' scores matmuls use PE row-groups 0-63 / 64-127 and run
    concurrently.
  - All matmul inputs fp16, fp32 PSUM accumulation.
  - ScalarE runs exp ONLY; every PSUM drain is on VectorE.
  - PSUM: scp 2x2 banks + av 2 + one uniform fill tag (2x1 banks) = 8.
  - DMA: inputs stream as full-row tiles (>=2KB packets); output written as
    [128, 1024] f16 tiles (2KB rows) to keep DMA dispatch count low.
"""

import contextlib
import itertools

import numpy as np

import concourse.bacc as bacc
import concourse.mybir as mybir
import concourse.tile as tile
from concourse.bass_utils import run_bass_kernel_spmd

F16 = mybir.dt.float16
F32 = mybir.dt.float32
AF = mybir.ActivationFunctionType

B, S, D, H, HD = 4, 2048, 1024, 16, 64
NQB = S // 512        # query blocks of 512

_cache = {}


class _Parts:
    """Shared emission helpers + tile handles."""

    def __init__(self, nc, ap, stream, fill, wrk, expool, kcs):
        self.nc = nc
        self.ap = ap
        self.stream = stream
        self.fill = fill
        self.wrk = wrk
        self.expool = expool
        self.kcs = kcs

    def q_chain(self, b, qb, xq):
        """One Q projection chain (8 MMs) + drain; yields PE-cost quanta.
        xq = (tiles, col0): tiles hold columns starting at col0."""
        nc = self.nc
        xq_t, col0 = xq
        psq = self.fill.tile([128, 512], F32, tag="fill", name="psq", bufs=2)
        for dj2 in range(4):
            for dj in (2 * dj2, 2 * dj2 + 1):
                nc.tensor.matmul(
                    psq[:], self.wq_sb[dj][:],
                    xq_t[dj][:, qb * 512 - col0:(qb + 1) * 512 - col0],
                    start=(dj == 0), stop=(dj == 7))
            yield 426
        nc.vector.tensor_copy(
            self.qT_sb[b][:, qb * 512:(qb + 1) * 512], psq[:])

    def k_chain(self, b, c0, n, xk_t, col0):
        nc = self.nc
        psk = self.fill.tile([128, 512], F32, tag="fill", name="psk", bufs=2)
        for dj2 in range(4):
            for dj in (2 * dj2, 2 * dj2 + 1):
                nc.tensor.matmul(psk[:, 0:n], self.wk_sb[dj][:],
                                 xk_t[dj][:, c0 - col0:c0 - col0 + n],
                                 start=(dj == 0), stop=(dj == 7))
            yield 426
        nc.vector.tensor_copy(self.kT_sb[b][:, c0:c0 + n], psk[:, 0:n])

    def v_chain(self, b, sc, xv_t, col0):
        nc = self.nc
        off = sc * 128 - col0
        psv = self.fill.tile([128, 512], F32, tag="fill", name="psv", bufs=2)
        for dj2 in range(4):
            for dj in (2 * dj2, 2 * dj2 + 1):
                nc.tensor.matmul(psv[:, 0:128],
                                 xv_t[dj][:, off:off + 128],
                                 self.wv_sb[dj][:], start=(dj == 0),
                                 stop=(dj == 7))
            yield 107
        nc.vector.tensor_copy(
            self.v_sb[b][sc][:, :, 0:HD], psv[:, 0:128])
        nc.vector.tensor_copy(
            self.v_sb[b][sc][:, :, HD], self.vm_sb[:, b * 16 + sc, :])

    def load_x(self, name, b, col0, ncols, tag, w):
        """Stream tiles [128, w] (one per dj) holding cols col0:col0+ncols."""
        nc = self.nc
        ts = []
        for dj in range(8):
            t = self.stream.tile([128, w], F16, tag=tag,
                                 name=f"{name}t", bufs=8)
            nc.sync.dma_start(
                t[:, 0:ncols],
                self.ap[f"{name}{b}"][dj * 128:(dj + 1) * 128,
                                      col0:col0 + ncols])
            ts.append(t)
        return ts


def _gen_rest_proj(p, b):
    """K/V rounds beyond round0 and Q chains qb1-3 for batch b, as fill.
    K/V first so kT/v stay ahead of the attention kc consumer."""
    nk = p.kcs[b] * 128
    for r0 in range(1024, nk, 1024):
        n2 = min(1024, nk - r0)
        xk_t = p.load_x("xk", b, r0, n2, "xk", 1024)
        yield 0
        for c0 in range(r0, r0 + n2, 512):
            yield from p.k_chain(b, c0, min(512, r0 + n2 - c0), xk_t, r0)
        xv_t = p.load_x("xv", b, r0, n2, "xv", 1024)
        yield 0
        for sc in range(r0 // 128, (r0 + n2) // 128):
            yield from p.v_chain(b, sc, xv_t, r0)
    for qb in range(1, NQB):
        yield from p.q_chain(b, qb, p.xq_t[b])


def _gen_prologue(p, b, eager=False):
    """Round-0 K/V + Q(qb0) for batch b.  When eager (first batch), emitted
    directly (not via pull) with qb0's xq split out so the first chain
    starts after ~1MB of DMA instead of ~4MB."""
    nk = p.kcs[b] * 128
    n2 = min(1024, nk)
    if eager:
        xqa = (p.load_x("xq", b, 0, 512, "xqa", 512), 0)
        xk_t = p.load_x("xk", b, 0, n2, "xk", 1024)
        xv_t = p.load_x("xv", b, 0, n2, "xv", 1024)
        p.xq_t[b] = (p.load_x("xq", b, 512, S - 512, "xq", S), 512)
        yield 0
        yield from p.q_chain(b, 0, xqa)
    else:
        xk_t = p.load_x("xk", b, 0, n2, "xk", 1024)
        xv_t = p.load_x("xv", b, 0, n2, "xv", 1024)
        p.xq_t[b] = (p.load_x("xq", b, 0, S, "xq", S), 0)
        yield 0
        yield from p.q_chain(b, 0, p.xq_t[b])
    for c0 in range(0, n2, 512):
        yield from p.k_chain(b, c0, min(512, n2 - c0), xk_t, 0)
    for sc in range(0, n2 // 128):
        yield from p.v_chain(b, sc, xv_t, 0)


def _gen_o_proj(p, b):
    """Output projection partial for one batch; [128, 1024] f16 staging
    tiles keep the output DMA rows at 2KB.

    Gated on p.ctx_done[b]: a consumer instruction emitted BEFORE its
    producer gets no dependency edge from the tracker (reads only link to
    already-emitted writes), so each sc block must wait until the epilogue
    that writes ctx[:, sc*128:(sc+1)*128] has been emitted.  Yielding None
    tells pull() to stop this round and retry on the next pull."""
    nc = p.nc
    for sc in range(16):
        while p.ctx_done[b] <= sc // 4:
            yield None
        ot = p.wrk.tile([128, 1024], F16, tag="ot", name="ot", bufs=4)
        for ih in range(2):
            po = p.fill.tile([128, 512], F32, tag="fill", name="po", bufs=2)
            nc.tensor.matmul(
                po[:], p.ctx_sb[b][:, sc * 128:(sc + 1) * 128],
                p.wo_sb[0][:, ih * 512:(ih + 1) * 512],
                start=True, stop=True)
            yield 213
            nc.vector.tensor_copy(ot[:, ih * 512:(ih + 1) * 512], po[:])
        nc.sync.dma_start(
            p.ap["out"][b, sc * 128:(sc + 1) * 128, :], ot[:])


def _attention_b(p, b, psum, pull):
    """Attention for one batch (2 heads): ACT-bound pipeline over (qb, kc).
    ScalarE does exp only; fill quanta are pulled after each kc."""
    nc = p.nc
    kcb = p.kcs[b]
    for qb in range(NQB):
        av = psum.tile([HD + 1, 2, 512], F32, tag="av", name="av")
        for kc in range(kcb):
            scp = psum.tile([128, 2, 512], F32, tag="sc", name="scp", bufs=2)
            for h2 in range(2):
                nc.tensor.matmul(
                    scp[:, h2, :],
                    p.kT_sb[b][64 * h2:64 * h2 + 64,
                               kc * 128:(kc + 1) * 128],
                    p.qT_sb[b][64 * h2:64 * h2 + 64,
                               qb * 512:(qb + 1) * 512],
                    start=True, stop=True)
            ex = p.expool.tile([128, 2, 512], F16, tag="ex", name="ex")
            nc.scalar.activation(ex[:], scp[:], AF.Exp, scale=0.125)
            for h2 in range(2):
                nc.tensor.matmul(
                    av[:, h2, :], p.v_sb[b][kc][:, h2, :], ex[:, h2, :],
                    start=(kc == 0), stop=(kc == kcb - 1))
            pull(500)
        # Drain av to SBUF (frees the PSUM slot), then normalize off the
        # critical path: ctx[m, q] = av[m, q] / av[64, q].
        wrk = p.wrk
        avc = wrk.tile([HD + 1, 2, 512], F32, tag="avc", name="avc", bufs=2)
        nc.vector.tensor_copy(avc[:], av[:])
        # Move the sums row to partition 0 (engines can't read from a
        # non-32-aligned base partition; partition_broadcast reads p0 only).
        r0 = wrk.tile([1, 1024], F32, tag="r0", name="r0", bufs=2)
        nc.sync.dma_start(r0[:], avc[HD:HD + 1, :, :])
        bc = wrk.tile([HD, 1024], F32, tag="bc", name="bc", bufs=2)
        nc.gpsimd.partition_broadcast(bc[:], r0[0:1, :])
        recb = wrk.tile([HD, 1024], F32, tag="recb", name="recb", bufs=2)
        nc.vector.reciprocal_approx_fast(recb[:], bc[:])
        nc.vector.tensor_mul(
            p.ctx_sb[b][0:HD, qb * 512:(qb + 1) * 512],
            avc[0:HD, 0, :], recb[:, 0:512])
        tmp = wrk.tile([HD, 512], F16, tag="tmpb", name="tmp", bufs=2)
        nc.vector.tensor_mul(tmp[:], avc[0:HD, 1, :], recb[:, 512:1024])
        nc.sync.dma_start(
            p.ctx_sb[b][HD:128, qb * 512:(qb + 1) * 512], tmp[:])
        p.ctx_done[b] = qb + 1


def _emit(nc, tc, ap, kcs):
    es = contextlib.ExitStack()
    with es:
        const = es.enter_context(tc.tile_pool(name="const", bufs=1))
        resid = es.enter_context(tc.tile_pool(name="resid", bufs=1))
        stream = es.enter_context(tc.tile_pool(name="stream", bufs=3))
        expool = es.enter_context(tc.tile_pool(name="expool", bufs=3))
        wrk = es.enter_context(tc.tile_pool(name="wrk", bufs=2))

        order = sorted(range(B), key=lambda b: -kcs[b])

        with tc.tile_pool(name="fill_psum", bufs=1, space="PSUM") as fill:
            p = _Parts(nc, ap, stream, fill, wrk, expool, kcs)

            # constants: per-dj [din-chunk, dout=128] weight tiles
            p.wq_sb = [const.tile([128, 128], F16, tag=f"wq{i}",
                                  name=f"wq{i}") for i in range(8)]
            p.wk_sb = [const.tile([128, 128], F16, tag=f"wk{i}",
                                  name=f"wk{i}") for i in range(8)]
            p.wv_sb = [const.tile([128, 128], F16, tag=f"wv{i}",
                                  name=f"wv{i}") for i in range(8)]
            p.wo_sb = [const.tile([128, D], F16, tag="wo", name="wo")]
            p.vm_sb = const.tile([128, 64, 2], F16, tag="vmask", name="vmask")
            nc.sync.dma_start(p.vm_sb[:], ap["vones"])
            for i in range(8):
                nc.sync.dma_start(p.wq_sb[i][:],
                                  ap["wq"][i * 128:(i + 1) * 128, :])
                nc.sync.dma_start(p.wk_sb[i][:],
                                  ap["wk"][i * 128:(i + 1) * 128, :])
                nc.sync.dma_start(p.wv_sb[i][:],
                                  ap["wv"][i * 128:(i + 1) * 128, :])
            nc.sync.dma_start(p.wo_sb[0][:], ap["wo"])

            # residents (per batch)
            p.qT_sb = [resid.tile([128, S], F16, tag=f"qT{b}", name=f"qT{b}")
                       for b in range(B)]
            p.kT_sb = [resid.tile([128, kcs[b] * 128], F16, tag=f"kT{b}",
                                  name=f"kT{b}") for b in range(B)]
            p.ctx_sb = [resid.tile([128, S], F16, tag=f"ctx{b}",
                                   name=f"ctx{b}") for b in range(B)]
            p.v_sb = [[resid.tile([128, 2, HD + 1], F16, tag=f"v{b}_{i}",
                                  name=f"v{b}_{i}") for i in range(kcs[b])]
                      for b in range(B)]
            p.xq_t = [None] * B
            p.ctx_done = [0] * B

            # eager prologue: batch order[0]'s qb0 + K/V round 0
            for _ in _gen_prologue(p, order[0], eager=True):
                pass

            # global fill chain, pulled from the attention loops:
            #   rest(b_i) -> prologue(b_{i+1}) -> rest(b_{i+1}) emitted
            #   lazily; O(b_i) only becomes pullable after att(b_i) was
            #   emitted (guaranteed by pull order).
            parts = []
            parts.append(lambda: _gen_rest_proj(p, order[0]))
            for i in range(1, B):
                parts.append(lambda b=order[i]: _gen_prologue(p, b))
                parts.append(lambda b=order[i]: _gen_rest_proj(p, b))
                parts.append(lambda b=order[i - 1]: _gen_o_proj(p, b))
            parts.append(lambda: _gen_o_proj(p, order[B - 1]))
            fill_iter = itertools.chain.from_iterable(
                g() for g in parts)

            _done = object()

            def pull(budget):
                while budget > 0:
                    cost = next(fill_iter, _done)
                    if cost is _done:
                        return
                    if cost is None:
                        return  # producer not emitted yet; retry next pull
                    budget -= cost

            with tc.tile_pool(name="at_psum", bufs=1,
                              space="PSUM") as at_psum:
                for i, b in enumerate(order):
                    _attention_b(p, b, at_psum, pull)
                # drain remaining fill (final O-projections)
                pull(10 ** 9)


def _build(kcs):
    key = ("nc", tuple(kcs))
    if key in _cache:
        return _cache[key]
    nc = bacc.Bacc("TRN2", target_bir_lowering=False, debug=False, num_devices=8)
    ap = {"wq": nc.dram_tensor("wq", [D, 128], F16, kind="ExternalInput").ap(),
          "wk": nc.dram_tensor("wk", [D, 128], F16, kind="ExternalInput").ap(),
          "wv": nc.dram_tensor("wv", [D, 128], F16, kind="ExternalInput").ap(),
          "wo": nc.dram_tensor("wo", [128, D], F16, kind="ExternalInput").ap(),
          "vones": nc.dram_tensor("vones", [128, 64, 2], F16,
                                  kind="ExternalInput").ap(),
          "out": nc.dram_tensor("out", [B, S, D], F16,
                                kind="ExternalOutput").ap()}
    for b in range(B):
        ap[f"xq{b}"] = nc.dram_tensor(f"xq{b}", [D, S], F16,
                                      kind="ExternalInput").ap()
        ap[f"xk{b}"] = nc.dram_tensor(f"xk{b}", [D, kcs[b] * 128], F16,
                                      kind="ExternalInput").ap()
        ap[f"xv{b}"] = nc.dram_tensor(f"xv{b}", [D, kcs[b] * 128], F16,
                                      kind="ExternalInput").ap()
    with tile.TileContext(nc) as tc:
        _emit(nc, tc, ap, kcs)
    nc.compile()
    _cache[key] = nc
    return nc


def _in_maps(kcs, queries, keys, values, valid_len, W_q, W_k, W_v, W_o):
    f16 = np.float16
    # host-masked ones column: 1 where k < valid_len[b], else 0
    # vones[p, b*16+sc, h] = 1 if sc*128+p < valid_len[b] else 0
    kpos = np.arange(16 * 128).reshape(16, 128)
    vones = np.zeros((128, 64, 2), f16)
    for b in range(B):
        v1 = (kpos < int(valid_len[b])).astype(f16)  # [16, 128]
        vones[:, b * 16:(b + 1) * 16, :] = v1.T[:, :, None]
    maps = []
    for c in range(8):
        j0 = 128 * c
        m = {
            "wq": np.ascontiguousarray(W_q[j0:j0 + 128, :].T).astype(f16),
            "wk": np.ascontiguousarray(W_k[j0:j0 + 128, :].T).astype(f16),
            "wv": np.ascontiguousarray(W_v[j0:j0 + 128, :].T).astype(f16),
            "wo": np.ascontiguousarray(W_o[:, j0:j0 + 128].T).astype(f16),
            "vones": vones,
        }
        for b in range(B):
            nk = kcs[b] * 128
            xv = values[b][:nk].T.copy()      # [D, nk]
            xv[:, int(valid_len[b]):] = 0.0   # mask padding rows of v
            m[f"xq{b}"] = np.ascontiguousarray(queries[b].T).astype(f16)
            m[f"xk{b}"] = np.ascontiguousarray(keys[b][:nk].T).astype(f16)
            m[f"xv{b}"] = xv.astype(f16)
        maps.append(m)
    return maps


def kernel(queries, keys, values, valid_len, W_q, W_k, W_v, W_o, _run_kwargs=None):
    queries = np.asarray(queries, np.float32)
    keys = np.asarray(keys, np.float32)
    values = np.asarray(values, np.float32)
    valid_len = np.asarray(valid_len)
    W_q = np.asarray(W_q, np.float32)
    W_k = np.asarray(W_k, np.float32)
    W_v = np.asarray(W_v, np.float32)
    W_o = np.asarray(W_o, np.float32)

    kcs = [max(1, min(16, -(-int(valid_len[b]) // 128))) for b in range(B)]
    nc = _build(kcs)
    maps = _in_maps(kcs, queries, keys, values, valid_len, W_q, W_k, W_v, W_o)
    res = run_bass_kernel_spmd(nc, maps, list(range(8)), **(_run_kwargs or {}))
    out = np.zeros((B, S, D), np.float32)
    for c in range(8):
        out += res.results[c]["out"].astype(np.float32)
    if _run_kwargs:
        _cache["last_results"] = res
    return out
